# revision 9
# baseline (speedup 1.0000x reference)
"""Trainium2 Bass kernel for nn_ChEBIRecNN (recurrent DAG GNN).

Strategy: data-parallel over molecules (8 per core x 8 cores). Each core runs
the 31-level recurrence with h kept transposed [L=104, W=256] (features on
partitions). The per-level parent gather is executed on the TensorEngine as
one-hot matmuls: A = h_prev @ [W_p0 | W_p1] (2 MMs), then
C = sum_p OneHot_p^T @ A_p accumulated in PSUM together with the ctx matmul
(uniform K=128 accumulation group), finished by a Relu+bias activation.
One-hot matrices are built on the VectorEngine with tensor_scalar(is_equal)
against an iota column; parent indices are broadcast across partitions by DMA.
Everything on-chip is bf16 (fp32 PSUM accumulation); final mean/classifier/
sigmoid run on host in float64.
"""
import sys

sys.path.insert(0, "/opt/trn_rl_repo")

import numpy as np
import ml_dtypes

import concourse.bass as bass  # noqa: F401  (bass must import before bacc)
from concourse import bacc
import concourse.mybir as mybir
from concourse.tile import TileContext

BF16 = ml_dtypes.bfloat16
L = 104      # feature length
S = 500      # classes
M = 64       # molecules
T = 32       # levels
W = 256      # nodes per level
SK = 7       # sink parents
NCORES = 8
MPC = M // NCORES          # molecules per core
GL = 4                     # levels per ctx/idx chunk
NQ = T // GL               # chunks per molecule (ctx: 8 x 4 levels; idx same, last slot padded)

DT = mybir.dt


def build_nc():
    nc = bacc.Bacc(None, target_bir_lowering=False)

    d_ctx = nc.dram_tensor("ctx", [MPC, 128, T * W], DT.bfloat16, kind="ExternalInput")
    d_idx = nc.dram_tensor("idxb", [MPC, NQ, GL * 2 * W], DT.bfloat16, kind="ExternalInput")
    d_sidx = nc.dram_tensor("sidx", [MPC, 8], DT.bfloat16, kind="ExternalInput")
    d_wp = nc.dram_tensor("wp", [L, 2 * L], DT.bfloat16, kind="ExternalInput")
    d_wc = nc.dram_tensor("wc", [128, L], DT.bfloat16, kind="ExternalInput")
    d_ws = nc.dram_tensor("ws", [128, L], DT.bfloat16, kind="ExternalInput")
    d_w7 = nc.dram_tensor("w7", [128, SK * S], DT.bfloat16, kind="ExternalInput")
    d_id = nc.dram_tensor("ident", [L, L], DT.bfloat16, kind="ExternalInput")
    d_iota = nc.dram_tensor("iota", [128, 2], DT.float32, kind="ExternalInput")
    d_bs = nc.dram_tensor("bs", [L, 1], DT.float32, kind="ExternalInput")
    d_bi = nc.dram_tensor("bi", [L, 1], DT.float32, kind="ExternalInput")
    d_out = nc.dram_tensor("sink", [MPC, S], DT.float32, kind="ExternalOutput")

    with TileContext(nc) as tc:
        with tc.tile_pool(name="const", bufs=1) as cpool, \
             tc.tile_pool(name="ctxp", bufs=16) as ctxp, \
             tc.tile_pool(name="idxp", bufs=10) as idxp, \
             tc.tile_pool(name="sidxp", bufs=2) as sidxp, \
             tc.tile_pool(name="hp", bufs=2) as hp, \
             tc.tile_pool(name="ap", bufs=8) as ap_pool, \
             tc.tile_pool(name="ohp", bufs=10) as ohp, \
             tc.tile_pool(name="hnatp", bufs=2) as hnatp, \
             tc.tile_pool(name="pA", bufs=4, space="PSUM") as psA, \
             tc.tile_pool(name="pC", bufs=3, space="PSUM") as psC, \
             tc.tile_pool(name="pS", bufs=1, space="PSUM") as psS:

            t_wp = cpool.tile([L, 2 * L], DT.bfloat16)
            t_wc = cpool.tile([128, L], DT.bfloat16)
            t_ws = cpool.tile([128, L], DT.bfloat16)
            t_w7 = cpool.tile([128, SK * S], DT.bfloat16)
            t_id = cpool.tile([L, L], DT.bfloat16)
            t_iota = cpool.tile([128, 2], DT.float32)
            t_bs = cpool.tile([L, 1], DT.float32)
            t_bi = cpool.tile([L, 1], DT.float32)
            t_G = cpool.tile([128, SK * MPC], DT.bfloat16)   # [128, (sk, mol)]
            nc.sync.dma_start(out=t_wp[:], in_=d_wp[:])
            nc.sync.dma_start(out=t_wc[:], in_=d_wc[:])
            nc.sync.dma_start(out=t_ws[:], in_=d_ws[:])
            nc.sync.dma_start(out=t_w7[:], in_=d_w7[:])
            nc.sync.dma_start(out=t_id[:], in_=d_id[:])
            nc.sync.dma_start(out=t_iota[:], in_=d_iota[:])
            nc.sync.dma_start(out=t_bs[:], in_=d_bs[:])
            nc.sync.dma_start(out=t_bi[:], in_=d_bi[:])
            nc.vector.memset(t_G[:], 0.0)

            ctx_tiles = [{} for _ in range(MPC)]
            idx_tiles = [{} for _ in range(MPC)]
            oh_tiles = [{} for _ in range(MPC)]
            h_cur = [None] * MPC

            def get_ctx(m, q):
                if q not in ctx_tiles[m]:
                    t = ctxp.tile([128, GL * W], DT.bfloat16, tag="ctx")
                    nc.sync.dma_start(
                        out=t[:], in_=d_ctx[m, :, q * GL * W:(q + 1) * GL * W])
                    ctx_tiles[m][q] = t
                return ctx_tiles[m][q]

            def get_idx(m, q):
                if q not in idx_tiles[m]:
                    t = idxp.tile([128, GL * 2 * W], DT.bfloat16, tag="idx")
                    nc.gpsimd.dma_start(
                        out=t[:],
                        in_=d_idx[m, q:q + 1, :].to_broadcast((128, GL * 2 * W)))
                    idx_tiles[m][q] = t
                return idx_tiles[m][q]

            def get_oh(m, q):
                # one-hot for a whole 4-level chunk: [128, 2(jc) * GL * 512]
                if q not in oh_tiles[m]:
                    idx_t = get_idx(m, q)
                    CH = GL * 2 * W
                    t = ohp.tile([128, 2 * CH], DT.bfloat16, tag="oh")
                    for jc in range(2):
                        nc.vector.tensor_scalar(
                            out=t[:, jc * CH:(jc + 1) * CH], in0=idx_t[:],
                            scalar1=t_iota[:, jc:jc + 1], scalar2=None,
                            op0=mybir.AluOpType.is_equal)
                    oh_tiles[m][q] = t
                return oh_tiles[m][q]

            # level 0 for all molecules: h0 = relu(W_single.T @ ctx0T + b)
            for m in range(MPC):
                c0 = get_ctx(m, 0)
                p0 = psC.tile([L, W], DT.float32, tag="pC")
                nc.tensor.matmul(p0[:], t_ws[:], c0[:, 0:W], start=True, stop=True)
                h = hp.tile([L, W], DT.bfloat16, tag=f"h{m}")
                nc.scalar.activation(h[:], p0[:],
                                     mybir.ActivationFunctionType.Relu,
                                     bias=t_bs[:], scale=1.0)
                h_cur[m] = h

            # levels 1..31, molecules in lockstep
            for t in range(1, T):
                for m in range(MPC):
                    h = h_cur[m]
                    ctx_t = get_ctx(m, t // GL)
                    coff = (t % GL) * W
                    q = (t - 1) // GL
                    lq = (t - 1) % GL
                    t_oh = get_oh(m, q)
                    CH = GL * 2 * W
                    # staggered prefetch of next chunk (spread across levels)
                    if lq == m % GL:
                        if t // GL + 1 < NQ:
                            get_ctx(m, t // GL + 1)
                        if q + 1 < NQ:
                            get_oh(m, q + 1)

                    # A = h_prevT.T @ [Wp0|Wp1] -> [256(j), 208(p,l)], 2 chunks
                    t_A = ap_pool.tile([128, 416], DT.bfloat16, tag="A")
                    pA = psA.tile([128, 416], DT.float32, tag="pA")
                    for jc in range(2):
                        nc.tensor.matmul(pA[:, jc * 208:(jc + 1) * 208],
                                         h[:, jc * 128:(jc + 1) * 128],
                                         t_wp[:], start=True, stop=True)
                    # contiguous psum->sbuf copies, halves on ACT and DVE
                    nc.scalar.activation(t_A[:, 0:208], pA[:, 0:208],
                                         mybir.ActivationFunctionType.Copy)
                    nc.vector.tensor_copy(t_A[:, 208:416], pA[:, 208:416])

                    # C = sum_p OneHot_p^T A_p + W_ctx.T ctxT  (K=128, M=104 group)
                    pC = psC.tile([L, W], DT.float32, tag="pC")
                    for k in range(4):
                        jc, p = divmod(k, 2)
                        nc.tensor.matmul(
                            pC[:], t_A[:, jc * 208 + p * L: jc * 208 + (p + 1) * L],
                            t_oh[:, jc * CH + lq * 2 * W + p * W:
                                 jc * CH + lq * 2 * W + (p + 1) * W],
                            start=(k == 0), stop=False)
                    nc.tensor.matmul(pC[:], t_wc[:], ctx_t[:, coff:coff + W],
                                     start=False, stop=True)

                    h_new = hp.tile([L, W], DT.bfloat16, tag=f"h{m}")
                    nc.scalar.activation(h_new[:], pC[:],
                                         mybir.ActivationFunctionType.Relu,
                                         bias=t_bi[:], scale=1.0)
                    h_cur[m] = h_new

            for m in range(MPC):
                h = h_cur[m]
                # ---- sink for this molecule ----
                t_sx = sidxp.tile([128, 8], DT.bfloat16, tag="sidx")
                nc.sync.dma_start(out=t_sx[:], in_=d_sidx[m:m + 1, :].to_broadcast((128, 8)))

                # h natural via identity matmul: hnat[j, l] (2 chunks of 128 j)
                hnat = hnatp.tile([128, 2 * 128], DT.bfloat16, tag="hnat")
                for jc in range(2):
                    pN = psA.tile([128, L], DT.float32, tag="pA")
                    nc.tensor.matmul(pN[:], h[:, jc * 128:(jc + 1) * 128], t_id[:],
                                     start=True, stop=True)
                    nc.scalar.activation(hnat[:, jc * 128:jc * 128 + L], pN[:],
                                         mybir.ActivationFunctionType.Copy)

                # sink one-hot [256(j), 7(sk)] in 2 chunks; gather G = hnat.T @ oh
                t_so = sidxp.tile([128, 16], DT.bfloat16, tag="soh")
                for jc in range(2):
                    nc.vector.tensor_scalar(
                        out=t_so[:, jc * 8:jc * 8 + SK], in0=t_sx[:, 0:SK],
                        scalar1=t_iota[:, jc:jc + 1], scalar2=None,
                        op0=mybir.AluOpType.is_equal)
                pG = psC.tile([L, SK], DT.float32, tag="pC")
                for jc in range(2):
                    nc.tensor.matmul(pG[:], hnat[:, jc * 128:jc * 128 + L],
                                     t_so[:, jc * 8:jc * 8 + SK],
                                     start=(jc == 0), stop=(jc == 1))
                # G[l, sk] -> t_G[l, sk*8 + m]  (bf16, strided)
                nc.scalar.activation(
                    t_G[0:L, :].rearrange("p (sk b) -> p sk b", b=MPC)[:, :, m],
                    pG[:], mybir.ActivationFunctionType.Copy)

            # sink matmul: out[mol, s] = sum_sk sum_l G[l, (sk, mol)] * W7[(sk,l), s]
            pS = psS.tile([MPC, S], DT.float32, tag="pS")
            for sk in range(SK):
                nc.tensor.matmul(pS[:], t_G[:, sk * MPC:(sk + 1) * MPC],
                                 t_w7[:, sk * S:(sk + 1) * S],
                                 start=(sk == 0), stop=(sk == SK - 1))
            t_out = cpool.tile([MPC, S], DT.float32)
            nc.vector.tensor_copy(t_out[:], pS[:])
            nc.sync.dma_start(out=d_out[:], in_=t_out[:])

    nc.compile()
    nc.finalize()
    return nc


_CACHE = {}


def _get_state():
    if "nc" not in _CACHE:
        _CACHE["nc"] = build_nc()
    return _CACHE["nc"]


def _prep_inputs(inputs):
    context = np.asarray(inputs["context"], np.float32)
    parent_idx = np.asarray(inputs["parent_idx"], np.int32)
    sink_parent_idx = np.asarray(inputs["sink_parent_idx"], np.int32)
    W_single = np.asarray(inputs["W_single"], np.float32)
    W_int2 = np.asarray(inputs["W_int2"], np.float32)
    W_sink7 = np.asarray(inputs["W_sink7"], np.float32)

    # ctx transposed+padded: [M, 128, T*W], rows 104:128 zero
    ctx = np.zeros((M, 128, T * W), dtype=BF16)
    ctx[:, :L, :] = np.ascontiguousarray(
        np.transpose(context, (0, 3, 1, 2))).reshape(M, L, T * W).astype(BF16)

    # parent idx as bf16 rows [(p, w)] per level, padded to NQ*GL levels
    idxb = np.zeros((M, NQ, GL * 2 * W), dtype=BF16)
    pidx = np.transpose(parent_idx, (0, 1, 3, 2)).reshape(M, T - 1, 2 * W)  # [(p, w)]
    idxb.reshape(M, NQ * GL, 2 * W)[:, :T - 1] = pidx.astype(BF16)

    sidx = np.zeros((M, 8), dtype=BF16)
    sidx[:, :SK] = sink_parent_idx.astype(BF16)

    wp = np.ascontiguousarray(
        np.concatenate([W_int2[L:2 * L], W_int2[2 * L:]], axis=1)).astype(BF16)
    wc = np.zeros((128, L), dtype=BF16)
    wc[:L] = W_int2[:L].astype(BF16)
    ws = np.zeros((128, L), dtype=BF16)
    ws[:L] = W_single.astype(BF16)
    w7 = np.zeros((128, SK * S), dtype=BF16)
    w7[:L] = np.ascontiguousarray(
        np.transpose(W_sink7.reshape(SK, L, S), (1, 0, 2))).reshape(L, SK * S).astype(BF16)
    ident = np.eye(L, dtype=BF16)
    iota = np.stack([np.arange(128), np.arange(128, 256)], axis=1).astype(np.float32)
    bs = np.asarray(inputs["b_single"], np.float32).reshape(L, 1)
    bi = np.asarray(inputs["b_int2"], np.float32).reshape(L, 1)

    in_maps = []
    for c in range(NCORES):
        sl = slice(c * MPC, (c + 1) * MPC)
        in_maps.append({
            "ctx": ctx[sl], "idxb": idxb[sl], "sidx": sidx[sl],
            "wp": wp, "wc": wc, "ws": ws, "w7": w7, "ident": ident,
            "iota": iota, "bs": bs, "bi": bi,
        })
    return in_maps


def _finish(results, inputs):
    sink = np.concatenate([results[c]["sink"] for c in range(NCORES)], axis=0)
    avg = sink.astype(np.float64).mean(axis=0) + np.asarray(inputs["b_sink7"], np.float64)
    logits = avg @ np.asarray(inputs["W_cls"], np.float64) + np.asarray(inputs["b_cls"], np.float64)
    return (1.0 / (1.0 + np.exp(-logits))).astype(np.float32)


def _run_cached(nc, in_maps):
    """Compile-once runner (mirrors run_bass_via_pjrt multi-core path)."""
    import jax
    import jax.numpy as jnp  # noqa: F401
    from jax.sharding import Mesh, PartitionSpec
    from jax.experimental.shard_map import shard_map
    from concourse import bass2jax
    from concourse.bass2jax import _bass_exec_p, partition_id_tensor
    import concourse.mybir as mybir_

    if "sharded" not in _CACHE:
        bass2jax.install_neuronx_cc_hook()
        in_names, out_names, out_avals, zero_outs = [], [], [], []
        partition_name = nc.partition_id_tensor.name if nc.partition_id_tensor else None
        for alloc in nc.m.functions[0].allocations:
            if not isinstance(alloc, mybir_.MemoryLocationSet):
                continue
            name = alloc.memorylocations[0].name
            if alloc.kind == "ExternalInput":
                if name != partition_name:
                    in_names.append(name)
            elif alloc.kind == "ExternalOutput":
                out_names.append(name)
                shape = tuple(alloc.tensor_shape)
                dtype = mybir_.dt.np(alloc.dtype)
                out_avals.append(jax.core.ShapedArray(shape, dtype))
                zero_outs.append(np.zeros(shape, dtype))
        n_params = len(in_names)
        n_outs = len(out_avals)
        all_in_names = list(in_names) + list(out_names)
        if partition_name is not None:
            all_in_names.append(partition_name)
        donate = tuple(range(n_params, n_params + n_outs))

        def _body(*args):
            operands = list(args)
            if partition_name is not None:
                operands.append(partition_id_tensor())
            outs = _bass_exec_p.bind(
                *operands,
                out_avals=tuple(out_avals),
                in_names=tuple(all_in_names),
                out_names=tuple(out_names),
                lowering_input_output_aliases=(),
                sim_require_finite=True,
                sim_require_nnan=True,
                nc=nc,
            )
            return tuple(outs)

        devices = jax.devices()[:NCORES]
        mesh = Mesh(np.asarray(devices), ("core",))
        in_specs = (PartitionSpec("core"),) * (n_params + n_outs)
        out_specs = (PartitionSpec("core"),) * n_outs
        sharded = jax.jit(
            shard_map(_body, mesh=mesh, in_specs=in_specs, out_specs=out_specs,
                      check_rep=False),
            donate_argnums=donate, keep_unused=True)
        _CACHE["sharded"] = (sharded, in_names, out_names, out_avals, zero_outs)

    sharded, in_names, out_names, out_avals, zero_outs = _CACHE["sharded"]
    concat_in = [np.concatenate([np.asarray(m[nm]) for m in in_maps], axis=0)
                 for nm in in_names]
    concat_zeros = [np.zeros((NCORES * z.shape[0], *z.shape[1:]), z.dtype)
                    for z in zero_outs]
    out_arrs = sharded(*concat_in, *concat_zeros)
    return [
        {nm: np.asarray(out_arrs[i]).reshape(NCORES, *out_avals[i].shape)[c]
         for i, nm in enumerate(out_names)}
        for c in range(NCORES)
    ]


def kernel(**inputs) -> np.ndarray:
    nc = _get_state()
    in_maps = _prep_inputs(inputs)
    results = _run_cached(nc, in_maps)
    return _finish(results, inputs)


def run_traced(inputs):
    """Test-only: run via run_bass_kernel_spmd with tracing; returns
    (output, BassKernelResults)."""
    from concourse.bass_utils import run_bass_kernel_spmd
    nc = _get_state()
    in_maps = _prep_inputs(inputs)
    res = run_bass_kernel_spmd(nc, in_maps, list(range(NCORES)), trace=True)
    return _finish(res.results, inputs), res


# revision 10
# speedup vs baseline: 1.1531x; 1.1531x over previous
"""Trainium2 Bass kernel for nn_ChEBIRecNN (recurrent DAG GNN).

Strategy: data-parallel over molecules (8 per core x 8 cores). Each core runs
the 31-level recurrence with h kept transposed [L=104, W=256] (features on
partitions). The per-level parent gather is executed on the TensorEngine as
one-hot matmuls: A = h_prev @ [W_p0 | W_p1] (2 MMs), then
C = sum_p OneHot_p^T @ A_p accumulated in PSUM together with the ctx matmul
(uniform K=128 accumulation group), finished by a Relu+bias activation.
One-hot matrices are built on the VectorEngine with tensor_scalar(is_equal)
against an iota column; parent indices are broadcast across partitions by DMA.
Everything on-chip is bf16 (fp32 PSUM accumulation); final mean/classifier/
sigmoid run on host in float64.
"""
import sys

sys.path.insert(0, "/opt/trn_rl_repo")

import numpy as np
import ml_dtypes

import concourse.bass as bass  # noqa: F401  (bass must import before bacc)
from concourse import bacc
import concourse.mybir as mybir
from concourse.tile import TileContext

BF16 = ml_dtypes.bfloat16
L = 104      # feature length
S = 500      # classes
M = 64       # molecules
T = 32       # levels
W = 256      # nodes per level
SK = 7       # sink parents
NCORES = 8
MPC = M // NCORES          # molecules per core
GL = 4                     # levels per ctx/idx chunk
NQ = T // GL               # chunks per molecule (ctx: 8 x 4 levels; idx same, last slot padded)

DT = mybir.dt


def build_nc():
    nc = bacc.Bacc(None, target_bir_lowering=False)

    d_ctx = nc.dram_tensor("ctx", [MPC, 128, T * W], DT.bfloat16, kind="ExternalInput")
    d_idx = nc.dram_tensor("idxb", [MPC, NQ, GL * 2 * W], DT.bfloat16, kind="ExternalInput")
    d_sidx = nc.dram_tensor("sidx", [MPC, 8], DT.bfloat16, kind="ExternalInput")
    d_wp = nc.dram_tensor("wp", [L, 2 * L], DT.bfloat16, kind="ExternalInput")
    d_wc = nc.dram_tensor("wc", [128, L], DT.bfloat16, kind="ExternalInput")
    d_ws = nc.dram_tensor("ws", [128, L], DT.bfloat16, kind="ExternalInput")
    d_w7 = nc.dram_tensor("w7", [128, SK * S], DT.bfloat16, kind="ExternalInput")
    d_id = nc.dram_tensor("ident", [L, L], DT.bfloat16, kind="ExternalInput")
    d_iota = nc.dram_tensor("iota", [128, 2], DT.float32, kind="ExternalInput")
    d_bs = nc.dram_tensor("bs", [L, 1], DT.float32, kind="ExternalInput")
    d_bi = nc.dram_tensor("bi", [L, 1], DT.float32, kind="ExternalInput")
    d_out = nc.dram_tensor("sink", [MPC, S], DT.float32, kind="ExternalOutput")

    with TileContext(nc) as tc:
        with tc.tile_pool(name="const", bufs=1) as cpool, \
             tc.tile_pool(name="ctxp", bufs=16) as ctxp, \
             tc.tile_pool(name="idxp", bufs=10) as idxp, \
             tc.tile_pool(name="sidxp", bufs=2) as sidxp, \
             tc.tile_pool(name="hp", bufs=2) as hp, \
             tc.tile_pool(name="ap", bufs=8) as ap_pool, \
             tc.tile_pool(name="ohp", bufs=10) as ohp, \
             tc.tile_pool(name="hnatp", bufs=2) as hnatp, \
             tc.tile_pool(name="pA", bufs=4, space="PSUM") as psA, \
             tc.tile_pool(name="pC", bufs=3, space="PSUM") as psC, \
             tc.tile_pool(name="pS", bufs=1, space="PSUM") as psS:

            t_wp = cpool.tile([L, 2 * L], DT.bfloat16)
            t_wc = cpool.tile([128, L], DT.bfloat16)
            t_ws = cpool.tile([128, L], DT.bfloat16)
            t_w7 = cpool.tile([128, SK * S], DT.bfloat16)
            t_id = cpool.tile([L, L], DT.bfloat16)
            t_iota = cpool.tile([128, 2], DT.float32)
            t_bs = cpool.tile([L, 1], DT.float32)
            t_bi = cpool.tile([L, 1], DT.float32)
            t_G = cpool.tile([128, SK * MPC], DT.bfloat16)   # [128, (sk, mol)]
            nc.sync.dma_start(out=t_wp[:], in_=d_wp[:])
            nc.sync.dma_start(out=t_wc[:], in_=d_wc[:])
            nc.sync.dma_start(out=t_ws[:], in_=d_ws[:])
            nc.sync.dma_start(out=t_w7[:], in_=d_w7[:])
            nc.sync.dma_start(out=t_id[:], in_=d_id[:])
            nc.sync.dma_start(out=t_iota[:], in_=d_iota[:])
            nc.sync.dma_start(out=t_bs[:], in_=d_bs[:])
            nc.sync.dma_start(out=t_bi[:], in_=d_bi[:])
            nc.vector.memset(t_G[:], 0.0)

            ctx_tiles = [{} for _ in range(MPC)]
            idx_tiles = [{} for _ in range(MPC)]
            oh_tiles = [{} for _ in range(MPC)]
            h_cur = [None] * MPC

            def get_ctx(m, q):
                if q not in ctx_tiles[m]:
                    t = ctxp.tile([128, GL * W], DT.bfloat16, tag="ctx")
                    nc.sync.dma_start(
                        out=t[:], in_=d_ctx[m, :, q * GL * W:(q + 1) * GL * W])
                    ctx_tiles[m][q] = t
                return ctx_tiles[m][q]

            def get_idx(m, q):
                if q not in idx_tiles[m]:
                    t = idxp.tile([128, GL * 2 * W], DT.bfloat16, tag="idx")
                    nc.gpsimd.dma_start(
                        out=t[:],
                        in_=d_idx[m, q:q + 1, :].to_broadcast((128, GL * 2 * W)))
                    idx_tiles[m][q] = t
                return idx_tiles[m][q]

            OHL = 2                      # levels per one-hot batch
            OCH = OHL * 2 * W            # 1024 cols per jc block

            def get_oh(m, hq):
                # one-hot for OHL levels: [128, 2(jc) * OCH]
                if hq not in oh_tiles[m]:
                    q, r = divmod(hq * OHL, GL)
                    idx_t = get_idx(m, q)
                    t = ohp.tile([128, 2 * OCH], DT.bfloat16, tag="oh")
                    for jc in range(2):
                        nc.vector.tensor_scalar(
                            out=t[:, jc * OCH:(jc + 1) * OCH],
                            in0=idx_t[:, r * 2 * W:r * 2 * W + OCH],
                            scalar1=t_iota[:, jc:jc + 1], scalar2=None,
                            op0=mybir.AluOpType.is_equal)
                    oh_tiles[m][hq] = t
                return oh_tiles[m][hq]

            # level 0 for all molecules: h0 = relu(W_single.T @ ctx0T + b)
            for m in range(MPC):
                c0 = get_ctx(m, 0)
                p0 = psC.tile([L, W], DT.float32, tag="pC")
                nc.tensor.matmul(p0[:], t_ws[:], c0[:, 0:W], start=True, stop=True)
                h = hp.tile([L, W], DT.bfloat16, tag=f"h{m}")
                nc.scalar.activation(h[:], p0[:],
                                     mybir.ActivationFunctionType.Relu,
                                     bias=t_bs[:], scale=1.0)
                h_cur[m] = h

            # levels 1..31, molecules in lockstep
            for t in range(1, T):
                for m in range(MPC):
                    h = h_cur[m]
                    ctx_t = get_ctx(m, t // GL)
                    coff = (t % GL) * W
                    hq = (t - 1) // OHL
                    lq = (t - 1) % OHL
                    t_oh = get_oh(m, hq)
                    CH = OCH
                    # staggered ctx prefetch (spread across levels)
                    if (t - 1) % GL == m % GL and t // GL + 1 < NQ:
                        get_ctx(m, t // GL + 1)

                    # A = h_prevT.T @ [Wp0|Wp1] -> [256(j), 208(p,l)], 2 chunks
                    t_A = ap_pool.tile([128, 416], DT.bfloat16, tag="A")
                    pA = psA.tile([128, 416], DT.float32, tag="pA")
                    for jc in range(2):
                        nc.tensor.matmul(pA[:, jc * 208:(jc + 1) * 208],
                                         h[:, jc * 128:(jc + 1) * 128],
                                         t_wp[:], start=True, stop=True)
                    # contiguous psum->sbuf copies, halves on ACT and DVE
                    nc.scalar.activation(t_A[:, 0:208], pA[:, 0:208],
                                         mybir.ActivationFunctionType.Copy)
                    nc.vector.tensor_copy(t_A[:, 208:416], pA[:, 208:416])

                    # C = sum_p OneHot_p^T A_p + W_ctx.T ctxT  (K=128, M=104 group)
                    pC = psC.tile([L, W], DT.float32, tag="pC")
                    for k in range(4):
                        jc, p = divmod(k, 2)
                        nc.tensor.matmul(
                            pC[:], t_A[:, jc * 208 + p * L: jc * 208 + (p + 1) * L],
                            t_oh[:, jc * CH + lq * 2 * W + p * W:
                                 jc * CH + lq * 2 * W + (p + 1) * W],
                            start=(k == 0), stop=False)
                    nc.tensor.matmul(pC[:], t_wc[:], ctx_t[:, coff:coff + W],
                                     start=False, stop=True)

                    h_new = hp.tile([L, W], DT.bfloat16, tag=f"h{m}")
                    nc.scalar.activation(h_new[:], pC[:],
                                         mybir.ActivationFunctionType.Relu,
                                         bias=t_bi[:], scale=1.0)
                    h_cur[m] = h_new

            for m in range(MPC):
                h = h_cur[m]
                # ---- sink for this molecule ----
                t_sx = sidxp.tile([128, 8], DT.bfloat16, tag="sidx")
                nc.sync.dma_start(out=t_sx[:], in_=d_sidx[m:m + 1, :].to_broadcast((128, 8)))

                # h natural via identity matmul: hnat[j, l] (2 chunks of 128 j)
                hnat = hnatp.tile([128, 2 * 128], DT.bfloat16, tag="hnat")
                for jc in range(2):
                    pN = psA.tile([128, L], DT.float32, tag="pA")
                    nc.tensor.matmul(pN[:], h[:, jc * 128:(jc + 1) * 128], t_id[:],
                                     start=True, stop=True)
                    nc.scalar.activation(hnat[:, jc * 128:jc * 128 + L], pN[:],
                                         mybir.ActivationFunctionType.Copy)

                # sink one-hot [256(j), 7(sk)] in 2 chunks; gather G = hnat.T @ oh
                t_so = sidxp.tile([128, 16], DT.bfloat16, tag="soh")
                for jc in range(2):
                    nc.vector.tensor_scalar(
                        out=t_so[:, jc * 8:jc * 8 + SK], in0=t_sx[:, 0:SK],
                        scalar1=t_iota[:, jc:jc + 1], scalar2=None,
                        op0=mybir.AluOpType.is_equal)
                pG = psC.tile([L, SK], DT.float32, tag="pC")
                for jc in range(2):
                    nc.tensor.matmul(pG[:], hnat[:, jc * 128:jc * 128 + L],
                                     t_so[:, jc * 8:jc * 8 + SK],
                                     start=(jc == 0), stop=(jc == 1))
                # G[l, sk] -> t_G[l, sk*8 + m]  (bf16, strided)
                nc.scalar.activation(
                    t_G[0:L, :].rearrange("p (sk b) -> p sk b", b=MPC)[:, :, m],
                    pG[:], mybir.ActivationFunctionType.Copy)

            # sink matmul: out[mol, s] = sum_sk sum_l G[l, (sk, mol)] * W7[(sk,l), s]
            pS = psS.tile([MPC, S], DT.float32, tag="pS")
            for sk in range(SK):
                nc.tensor.matmul(pS[:], t_G[:, sk * MPC:(sk + 1) * MPC],
                                 t_w7[:, sk * S:(sk + 1) * S],
                                 start=(sk == 0), stop=(sk == SK - 1))
            t_out = cpool.tile([MPC, S], DT.float32)
            nc.vector.tensor_copy(t_out[:], pS[:])
            nc.sync.dma_start(out=d_out[:], in_=t_out[:])

    nc.compile()
    nc.finalize()
    return nc


_CACHE = {}


def _get_state():
    if "nc" not in _CACHE:
        _CACHE["nc"] = build_nc()
    return _CACHE["nc"]


def _prep_inputs(inputs):
    context = np.asarray(inputs["context"], np.float32)
    parent_idx = np.asarray(inputs["parent_idx"], np.int32)
    sink_parent_idx = np.asarray(inputs["sink_parent_idx"], np.int32)
    W_single = np.asarray(inputs["W_single"], np.float32)
    W_int2 = np.asarray(inputs["W_int2"], np.float32)
    W_sink7 = np.asarray(inputs["W_sink7"], np.float32)

    # ctx transposed+padded: [M, 128, T*W], rows 104:128 zero
    ctx = np.zeros((M, 128, T * W), dtype=BF16)
    ctx[:, :L, :] = np.ascontiguousarray(
        np.transpose(context, (0, 3, 1, 2))).reshape(M, L, T * W).astype(BF16)

    # parent idx as bf16 rows [(p, w)] per level, padded to NQ*GL levels
    idxb = np.zeros((M, NQ, GL * 2 * W), dtype=BF16)
    pidx = np.transpose(parent_idx, (0, 1, 3, 2)).reshape(M, T - 1, 2 * W)  # [(p, w)]
    idxb.reshape(M, NQ * GL, 2 * W)[:, :T - 1] = pidx.astype(BF16)

    sidx = np.zeros((M, 8), dtype=BF16)
    sidx[:, :SK] = sink_parent_idx.astype(BF16)

    wp = np.ascontiguousarray(
        np.concatenate([W_int2[L:2 * L], W_int2[2 * L:]], axis=1)).astype(BF16)
    wc = np.zeros((128, L), dtype=BF16)
    wc[:L] = W_int2[:L].astype(BF16)
    ws = np.zeros((128, L), dtype=BF16)
    ws[:L] = W_single.astype(BF16)
    w7 = np.zeros((128, SK * S), dtype=BF16)
    w7[:L] = np.ascontiguousarray(
        np.transpose(W_sink7.reshape(SK, L, S), (1, 0, 2))).reshape(L, SK * S).astype(BF16)
    ident = np.eye(L, dtype=BF16)
    iota = np.stack([np.arange(128), np.arange(128, 256)], axis=1).astype(np.float32)
    bs = np.asarray(inputs["b_single"], np.float32).reshape(L, 1)
    bi = np.asarray(inputs["b_int2"], np.float32).reshape(L, 1)

    in_maps = []
    for c in range(NCORES):
        sl = slice(c * MPC, (c + 1) * MPC)
        in_maps.append({
            "ctx": ctx[sl], "idxb": idxb[sl], "sidx": sidx[sl],
            "wp": wp, "wc": wc, "ws": ws, "w7": w7, "ident": ident,
            "iota": iota, "bs": bs, "bi": bi,
        })
    return in_maps


def _finish(results, inputs):
    sink = np.concatenate([results[c]["sink"] for c in range(NCORES)], axis=0)
    avg = sink.astype(np.float64).mean(axis=0) + np.asarray(inputs["b_sink7"], np.float64)
    logits = avg @ np.asarray(inputs["W_cls"], np.float64) + np.asarray(inputs["b_cls"], np.float64)
    return (1.0 / (1.0 + np.exp(-logits))).astype(np.float32)


def _run_cached(nc, in_maps):
    """Compile-once runner (mirrors run_bass_via_pjrt multi-core path)."""
    import jax
    import jax.numpy as jnp  # noqa: F401
    from jax.sharding import Mesh, PartitionSpec
    from jax.experimental.shard_map import shard_map
    from concourse import bass2jax
    from concourse.bass2jax import _bass_exec_p, partition_id_tensor
    import concourse.mybir as mybir_

    if "sharded" not in _CACHE:
        bass2jax.install_neuronx_cc_hook()
        in_names, out_names, out_avals, zero_outs = [], [], [], []
        partition_name = nc.partition_id_tensor.name if nc.partition_id_tensor else None
        for alloc in nc.m.functions[0].allocations:
            if not isinstance(alloc, mybir_.MemoryLocationSet):
                continue
            name = alloc.memorylocations[0].name
            if alloc.kind == "ExternalInput":
                if name != partition_name:
                    in_names.append(name)
            elif alloc.kind == "ExternalOutput":
                out_names.append(name)
                shape = tuple(alloc.tensor_shape)
                dtype = mybir_.dt.np(alloc.dtype)
                out_avals.append(jax.core.ShapedArray(shape, dtype))
                zero_outs.append(np.zeros(shape, dtype))
        n_params = len(in_names)
        n_outs = len(out_avals)
        all_in_names = list(in_names) + list(out_names)
        if partition_name is not None:
            all_in_names.append(partition_name)
        donate = tuple(range(n_params, n_params + n_outs))

        def _body(*args):
            operands = list(args)
            if partition_name is not None:
                operands.append(partition_id_tensor())
            outs = _bass_exec_p.bind(
                *operands,
                out_avals=tuple(out_avals),
                in_names=tuple(all_in_names),
                out_names=tuple(out_names),
                lowering_input_output_aliases=(),
                sim_require_finite=True,
                sim_require_nnan=True,
                nc=nc,
            )
            return tuple(outs)

        devices = jax.devices()[:NCORES]
        mesh = Mesh(np.asarray(devices), ("core",))
        in_specs = (PartitionSpec("core"),) * (n_params + n_outs)
        out_specs = (PartitionSpec("core"),) * n_outs
        sharded = jax.jit(
            shard_map(_body, mesh=mesh, in_specs=in_specs, out_specs=out_specs,
                      check_rep=False),
            donate_argnums=donate, keep_unused=True)
        _CACHE["sharded"] = (sharded, in_names, out_names, out_avals, zero_outs)

    sharded, in_names, out_names, out_avals, zero_outs = _CACHE["sharded"]
    concat_in = [np.concatenate([np.asarray(m[nm]) for m in in_maps], axis=0)
                 for nm in in_names]
    concat_zeros = [np.zeros((NCORES * z.shape[0], *z.shape[1:]), z.dtype)
                    for z in zero_outs]
    out_arrs = sharded(*concat_in, *concat_zeros)
    return [
        {nm: np.asarray(out_arrs[i]).reshape(NCORES, *out_avals[i].shape)[c]
         for i, nm in enumerate(out_names)}
        for c in range(NCORES)
    ]


def kernel(**inputs) -> np.ndarray:
    nc = _get_state()
    in_maps = _prep_inputs(inputs)
    results = _run_cached(nc, in_maps)
    return _finish(results, inputs)


def run_traced(inputs):
    """Test-only: run via run_bass_kernel_spmd with tracing; returns
    (output, BassKernelResults)."""
    from concourse.bass_utils import run_bass_kernel_spmd
    nc = _get_state()
    in_maps = _prep_inputs(inputs)
    res = run_bass_kernel_spmd(nc, in_maps, list(range(NCORES)), trace=True)
    return _finish(res.results, inputs), res
